# revision 80
# baseline (speedup 1.0000x reference)
"""Trainium2 Bass kernel for nn_DepthWiseSepConv (depthwise 5x5 + BN+hardswish
+ pointwise 1x1 + squeeze-excite gating + BN), data-parallel over batch on
8 NeuronCores.

Self-contained: hardcodes all shapes from the problem spec.

Per-core layout strategy (B_loc = 8 images per core), all-bf16 matmul path:
  - Host pre-packs x into [112=(4c,28h), 60g, 8b, 32w] bf16 with zero pad
    baked in, and the depthwise weights into expanded block-diagonal
    Toeplitz matrices [112=(4c,28hin), 60g, 5dx, 112=(4c,28hout)] bf16.
    Slab DMAs (6 groups each) keep transfers large and DMA count low
    (each DMA costs ~630ns on the shared HWDGE unit).
  - Depthwise conv per group: 5 PSUM-accumulated matmuls (one per kernel
    column dx) contracting (4c,28h_in).
  - BN1+relu(x+3) fused into the PSUM-draining activation (bf16 out);
    hardswish completes AFTER the T1 transpose so the PSUM->SBUF copy is
    absorbed into the min/mult ops (Pool min, DVE mult writing ActT).
  - T2 transposes to channel-major PWrhs[c, (b,h,w)] bf16 + DVE copies.
  - SE: DVE free-dim reduces (per ch, per 4-batch), two small matmuls,
    hardswish; gate FOLDED into the pointwise epilogue's per-partition
    scale/bias (sg = s2*g, sbg = s2*pw_b*g + t2), so phase D is just
    matmul + one Activation + batched stores (b-pairs).
"""

import sys

sys.path.insert(0, "/opt/trn_rl_repo")

import numpy as np
import ml_dtypes

import concourse.bass as bass
import concourse.mybir as mybir
import concourse.tile as tile
from concourse import bacc
from concourse.bass_utils import run_bass_kernel_spmd
from concourse.masks import make_identity

# ---------------------------------------------------------------- constants
N_CORES = 8
B, C, H, W = 64, 240, 28, 28
NB = B // N_CORES          # images per core
KK = 5                      # depthwise kernel size
G = C // 4                  # channel groups of 4 -> 60
R = 60                      # SE reduction dim
Cout = 240
HW = H * W                  # 784
WP = 32                     # padded W in SBUF x tiles (w in [-2, 30))
P = 112                     # output partitions for DW: (4c, 28h)
KP = 114                    # contraction partitions: (4c,28h) + 2 bias rows
                            # (bias split hi+lo across two bf16 rows)
EPS = 1e-5

CFG = {
    "slab": 10,         # groups per x/toep DMA (aligned to ch halves)
    "nrot": 4,          # rotation depth of slab buffers
    "dwbufs": 2,        # DW psum pool depth (quad tiles, 2 banks each)
    "t1bufs": 2,
    "t2bufs": 2,
    "pdbufs": 4,
    "store_split": False,   # y stores per (b, mo) instead of b-pairs
    # engine for the hardswish min op; must read PSUM, so pool is NOT
    # HW-legal (GPSIMD cannot access PSUM — walrus birverifier rejects it)
    "min_engine": "dve",
    "red_split": True,      # split SE reduce per (ch, q)
    "epi_dve": 3,           # every k-th D epilogue goes to DVE (1000=never)
    "bq_pops": 2,           # ch-0 tail work items emitted per ch-1 block
    "warmup": 10,           # dummy PE transposes during cold-start DMA wait
    "pw_head": True,        # emit 4 early PW tiles before SE(0)
    "stop_after": "",       # debug: ""|"a"|"ab"|"abc"
}

F32 = mybir.dt.float32
BF16 = mybir.dt.bfloat16
AL = mybir.AluOpType
ACT = mybir.ActivationFunctionType

# packed f32 const columns
NC_BN1S = 0          # [112, 60]
NC_BN1B = 60         # [112, 60]
NC_SE1L = 120        # [120, 2*60]
NC_SE1B = 240        # [60, 1]
NC_SE2L = 241        # [60, 2*120]
NC_SE2B = 481        # [120, 2]
NC_BN2S = 483        # [120, 2]
NC_BN2SB = 485       # [120, 2]
NC_BN2T = 487        # [120, 2]
NCOLS = 489


# ---------------------------------------------------------------- builder
_BUILD_CACHE = {}


def build_nc(cfg_key=None):
    cfg = dict(CFG)
    if cfg_key is not None:
        cfg.update(cfg_key)
    key = tuple(sorted(cfg.items()))
    if key in _BUILD_CACHE:
        return _BUILD_CACHE[key]

    SLAB = cfg["slab"]
    NSLAB = G // SLAB
    NROT = cfg["nrot"]

    nc = bacc.Bacc("TRN2", target_bir_lowering=False, debug=False,
                   num_devices=N_CORES)

    x_p = nc.declare_dram_parameter("xr", [KP, G, NB, WP], BF16,
                                    isOutput=False)
    toep_p = nc.declare_dram_parameter("toep", [KP, G, KK, P], BF16,
                                       isOutput=False)
    cpk_p = nc.declare_dram_parameter("cpk", [128, NCOLS], F32, isOutput=False)
    pwl_p = nc.declare_dram_parameter("pwl", [120, 2, 2, 120], BF16,
                                      isOutput=False)
    y_p = nc.declare_dram_parameter("y", [NB, Cout, H, W], F32, isOutput=True)

    with tile.TileContext(nc) as tc:
        cst = tc.alloc_tile_pool(name="cst", bufs=1)
        pers = tc.alloc_tile_pool(name="pers", bufs=1)

        # first compute slab's inputs go first (PE cold start gates on them);
        # pwl is not needed until phase D
        xs0 = None
        cpk = cst.tile([128, NCOLS], F32)
        pwl_sb = cst.tile([120, 2, 2, 120], BF16)
        ident = cst.tile([128, 128], BF16)

        def cc(col, ncol=1, rows=128):
            return cpk[0:rows, col:col + ncol]

        # persistent activation buffers
        # ActT[ch]: [(4b,28w)=112, 2q, (30gl, 4c, 28h) = 3360]
        ActT = [pers.tile([P, 2, 30 * P], BF16, name=f"actt_{ch}")
                for ch in range(2)]
        # PWrhs[ch]: [c=120, 8b, 28h, 28w]
        PWrhs = [pers.tile([120, NB, H, W], BF16, name=f"pwrhs_{ch}")
                 for ch in range(2)]
        PWflat = [t[:].rearrange("p b h w -> p (b h w)") for t in PWrhs]
        s_sb = [pers.tile([120, NB], F32, name=f"s_{ch}") for ch in range(2)]
        g_t = pers.tile([120, 2, NB], F32, name="gate")
        sg = pers.tile([120, 2, NB], F32, name="sg")
        sbg = pers.tile([120, 2, NB], F32, name="sbg")

        # slab rotation buffers
        xb = [pers.tile([KP, SLAB, NB, WP], BF16, name=f"x_rot{i}")
              for i in range(NROT)]
        tb = [pers.tile([KP, SLAB, KK, P], BF16, name=f"t_rot{i}")
              for i in range(NROT)]

        min_eng = nc.gpsimd if cfg["min_engine"] == "pool" else nc.vector

        t2state = {"pools": None, "i": 0}

        def phase_b_chunk(ch, q, h0, copy_eng=None):
            """T2: transpose ActT (ch, q, 4 h-rows) into PWrhs[ch]."""
            v = ActT[ch][:].rearrange("p q (gl c e) -> p q gl c e",
                                      gl=30, c=4)
            pools = t2state["pools"] or [t2ps]
            pool = pools[t2state["i"] % len(pools)]
            t2state["i"] += 1
            tp2 = pool.tile([120, 4, P], BF16, tag="t2")
            for hi in range(4):
                nc.tensor.transpose(tp2[:, hi, :],
                                    v[:, q, :, :, h0 + hi],
                                    ident[:P, :P])
            dst = PWrhs[ch][:, 4 * q:4 * q + 4, h0:h0 + 4, :]
            src = tp2[:].rearrange("p hh (b w) -> p b hh w", b=4)
            if copy_eng == "alt":
                copy_eng = "act" if (h0 // 4) % 2 == 0 else None
            if copy_eng == "act":
                nc.scalar.copy(dst, src)
            else:
                nc.vector.tensor_copy(dst, src)

        def phase_b(ch, q, copy_eng=None):
            for h0 in range(0, H, 4):
                phase_b_chunk(ch, q, h0, copy_eng)

        rtree = pers.tile([120, NB, 98], F32, name="rtree")
        tree_pend = []

        def reduce_b(ch, b, pool_tree=False):
            row = PWflat[ch].rearrange("p (b f) -> p b f", b=NB)[:, b, :]
            if pool_tree:
                # log-fold on the (otherwise idle) Pool engine — it cannot
                # touch PSUM, but these are all-SBUF. The small DVE finishing
                # reduce is DEFERRED (flush_trees) so a slow tree never
                # head-blocks DVE's in-order queue mid-phase-A.
                f1 = pa.tile([120, 392], F32, tag="rf1")
                nc.gpsimd.tensor_tensor(f1[:], row[:, 0:392], row[:, 392:784],
                                        AL.add)
                f2 = pa.tile([120, 196], F32, tag="rf2")
                nc.gpsimd.tensor_tensor(f2[:], f1[:, 0:196], f1[:, 196:392],
                                        AL.add)
                nc.gpsimd.tensor_tensor(rtree[:, b, :], f2[:, 0:98],
                                        f2[:, 98:196], AL.add)
                tree_pend.append((ch, b))
            else:
                nc.vector.tensor_reduce(s_sb[ch][:, b:b + 1], row,
                                        mybir.AxisListType.X, AL.add)

        def flush_trees():
            while tree_pend:
                ch, b = tree_pend.pop(0)
                nc.vector.tensor_reduce(s_sb[ch][:, b:b + 1], rtree[:, b, :],
                                        mybir.AxisListType.X, AL.add)

        def reduce_q(ch, q, pool_last=False):
            if cfg["red_split"]:
                for i, b in enumerate(range(4 * q, 4 * q + 4)):
                    reduce_b(ch, b, pool_tree=(pool_last and i == 0))
            else:
                nc.vector.tensor_reduce(
                    s_sb[ch][:, 4 * q:4 * q + 4],
                    PWflat[ch].rearrange("p (b f) -> p b f", b=NB)[
                        :, 4 * q:4 * q + 4, :],
                    mybir.AxisListType.X, AL.add)

        def se_q(q, half=None):
            """SE gate + fold into per-(mo,b) scale/bias for phase D.

            half=0/1 computes just a b-pair (lets D start as soon as the
            first two reduces land)."""
            if half is None:
                b0, nb = 4 * q, 4
            else:
                b0, nb = 4 * q + 2 * half, 2
            bs = slice(b0, b0 + nb)
            ps1 = seps.tile([120, 4], F32, tag="se")
            for ch in range(2):
                nc.tensor.matmul(ps1[0:R, 0:nb], cc(NC_SE1L + R * ch, R, 120),
                                 s_sb[ch][:, bs],
                                 start=(ch == 0), stop=(ch == 1))
            h1 = sep.tile([R, 4], F32, tag="seh")
            nc.scalar.activation(h1[:, 0:nb], ps1[0:R, 0:nb], ACT.Relu,
                                 bias=cc(NC_SE1B, 1, R))
            for mo in range(2):
                ps2 = seps.tile([120, 4], F32, tag="se")
                nc.tensor.matmul(ps2[:, 0:nb], cc(NC_SE2L + 120 * mo, 120, R),
                                 h1[:, 0:nb], start=True, stop=True)
                a2 = sep.tile([120, 4], F32, tag="sea")
                nc.scalar.activation(a2[:, 0:nb], ps2[:, 0:nb], ACT.Relu,
                                     bias=cc(NC_SE2B + mo, 1, 120))
                m2 = sep.tile([120, 4], F32, tag="sem")
                nc.vector.tensor_scalar(m2[:, 0:nb], a2[:, 0:nb],
                                        1.0 / 6.0, 1.0, AL.mult, AL.min)
                nc.vector.scalar_tensor_tensor(g_t[:, mo, bs], a2[:, 0:nb],
                                               3.0, m2[:, 0:nb],
                                               AL.subtract, AL.mult)
                nc.vector.tensor_scalar(sg[:, mo, bs], g_t[:, mo, bs],
                                        cc(NC_BN2S + mo, 1, 120), None,
                                        AL.mult)
                nc.vector.scalar_tensor_tensor(
                    sbg[:, mo, bs], g_t[:, mo, bs],
                    cc(NC_BN2SB + mo, 1, 120),
                    cc(NC_BN2T + mo, 1, 120).to_broadcast((120, nb)),
                    AL.mult, AL.add)

        def pw_tile(mo, b, nt):
            ps = pdps.tile([120, 392], F32, tag="pw")
            off = b * HW + nt * 392
            for kc in range(2):
                nc.tensor.matmul(ps[:], pwl_sb[:, kc, mo, :],
                                 PWflat[kc][:, off:off + 392],
                                 start=(kc == 0), stop=(kc == 1))
            return ps

        def epilogue(o_ap, ps, mo, b, eng):
            """o = ps*sg + sbg; alternates Act/DVE to balance the D phase."""
            if eng == "dve":
                nc.vector.scalar_tensor_tensor(
                    o_ap, ps[:], sg[:, mo, b:b + 1],
                    sbg[:, mo, b:b + 1].to_broadcast((120, 392)),
                    AL.mult, AL.add)
            else:
                nc.scalar.activation(o_ap, ps[:], ACT.Identity,
                                     bias=sbg[:, mo, b:b + 1],
                                     scale=sg[:, mo, b:b + 1])

        def phase_d(q, finer_stores=False, pw_head=None):
            for bp in (2 * q, 2 * q + 1):
                phase_d_bp(bp, pw_head)

        def phase_d_bp(bp, pw_head=None):
            """Pointwise conv + fused gate/BN2 epilogue + stores, one
            b-pair."""
            ei = 0
            if True:
                last = (bp == 3)
                for mo in range(2):
                    o2 = pd.tile([120, 2, HW], F32, tag="o2")
                    for bi in range(2):
                        b = 2 * bp + bi
                        for nt in range(2):
                            if pw_head and (mo, b, nt) in pw_head:
                                ps = pw_head.pop((mo, b, nt))
                            else:
                                ps = pw_tile(mo, b, nt)
                            # last pair: alternate engines so the final
                            # epilogues run in parallel
                            eng = ("dve" if ei % 2 else "act") if last else \
                                ("dve" if ei % cfg["epi_dve"] == 0
                                 else "act")
                            epilogue(o2[:, bi, nt * 392:(nt + 1) * 392], ps,
                                     mo, b, eng)
                            ei += 1
                        if last:
                            # final pair: store per image so the kernel-end
                            # drain isn't gated by one big DMA
                            y_ap = y_p[b, mo * 120:(mo + 1) * 120] \
                                .rearrange("c h w -> c (h w)")
                            nc.sync.dma_start(y_ap, o2[:, bi])
                    if not last:
                        y_ap = y_p[2 * bp:2 * bp + 2,
                                   mo * 120:(mo + 1) * 120].rearrange(
                            "b c h w -> c b (h w)")
                        nc.sync.dma_start(y_ap, o2[:])

        with tc.tile_pool(name="t2ps", bufs=cfg["t2bufs"], space="PSUM") \
                as t2ps, \
             tc.tile_pool(name="pa", bufs=3) as pa:

            # ============ Phase A: depthwise + relu + T1 + hardswish
            # Groups are processed in BLOCKS of 4 (pairs at the channel-half
            # edges) sharing one psum tile / one Act / one min / one
            # hardswish-mult (amortizes per-op access tax). BN1 is folded
            # into the toep weights + two all-ones x rows whose dx=2 weight
            # column carries the bias hi/lo, so the Act is a pure Relu with
            # no per-group scale/bias. T1 for block k-1 is emitted after
            # block k's matmuls (software pipelining). T2 chunks + reduces
            # for channel half 0 are spread across the second half's slabs.
            pend = []   # (g_start, nb, a_tile) awaiting T1 emission

            def emit_t1():
                g0, nb, a4 = pend.pop(0)
                ch, gl = (0, g0) if g0 < 30 else (1, g0 - 30)
                tp = t1ps.tile([P, 2, 4, P], BF16, tag="t1")
                for j in range(nb):
                    for q in range(2):
                        nc.tensor.transpose(tp[:, q, j, :],
                                            a4[:, j, 4 * q:4 * q + 4, :],
                                            ident[:P, :P])
                m = pa.tile([P, 2, 4, P], BF16, tag="m_g")
                min_eng.tensor_scalar(m[:, :, 0:nb, :], tp[:, :, 0:nb, :],
                                      1.0 / 6.0, 1.0, AL.mult, AL.min)
                nc.vector.scalar_tensor_tensor(
                    ActT[ch][:, :, gl * P:(gl + nb) * P].rearrange(
                        "p q (j e) -> p q j e", j=nb),
                    tp[:, :, 0:nb, :], 3.0, m[:, :, 0:nb, :],
                    AL.subtract, AL.mult)

            with tc.tile_pool(name="dwps", bufs=cfg["dwbufs"],
                              space="PSUM") as dwps, \
                 tc.tile_pool(name="t1ps", bufs=cfg["t1bufs"],
                              space="PSUM") as t1ps:
                # work items (T2 chunks + SE reduces for channel half 0)
                # interleaved into the second half's slabs
                bq = []
                for q in range(2):
                    bq += [("b", 0, q, h0) for h0 in range(0, H, 4)]
                    bq += [("r", 0, b, None) for b in range(4 * q, 4 * q + 4)]

                def pop_bq():
                    if bq:
                        kind, ch_, i1, i2 = bq.pop(0)
                        if kind == "b":
                            # copies on Act: a DVE copy here would head-block
                            # the min/stt chain that recycles t1ps for PE
                            phase_b_chunk(ch_, i1, i2, copy_eng="act")
                        else:
                            reduce_b(ch_, i1, pool_tree=True)

                for s in range(NSLAB):
                    xs = xb[s % NROT]
                    ts = tb[s % NROT]
                    g0 = SLAB * s
                    # pair-first on the slabs that start a channel half
                    # (cheap cold-start block), pair-last otherwise
                    blocks = [2, 4, 4] if s % (NSLAB // 2) == 0 else [4, 4, 2]
                    if s == 0:
                        # split the cold-start slab so block 0 is ready
                        # ASAP; consts wait until slab 2 (nothing in phase
                        # A reads them)
                        nc.sync.dma_start(xs[:, 0:2], x_p[:, 0:2])
                        nc.sync.dma_start(ts[:, 0:2], toep_p[:, 0:2])
                        nc.sync.dma_start(xs[:, 2:SLAB], x_p[:, 2:SLAB])
                        nc.sync.dma_start(ts[:, 2:SLAB], toep_p[:, 2:SLAB])
                        make_identity(nc, ident[:])
                        if cfg["warmup"]:
                            # ramp the PE p-state while the first slab's
                            # DMAs are in flight (results never read)
                            wt = t1ps.tile([P, 2, 4, P], BF16, tag="t1")
                            for _ in range(cfg["warmup"]):
                                nc.tensor.transpose(wt[:, 0, 0, :],
                                                    ident[:P, :P],
                                                    ident[:P, :P])
                    else:
                        # half-slab granularity: blocks only wait on the
                        # half they read, smoothing DMA arrival
                        h = SLAB // 2
                        for hs in range(2):
                            bs = slice(hs * h, (hs + 1) * h)
                            gs = slice(g0 + hs * h, g0 + (hs + 1) * h)
                            nc.sync.dma_start(xs[:, bs], x_p[:, gs])
                            nc.sync.dma_start(ts[:, bs], toep_p[:, gs])
                        if s == 2:
                            nc.sync.dma_start(cpk[:], cpk_p[:])
                            nc.sync.dma_start(pwl_sb[:], pwl_p[:])
                    boff = 0
                    for nb in blocks:
                        # regions padded to 256 f32 (half a PSUM bank) so no
                        # region crosses a bank; each bank's accumulation
                        # group is opened once (start on its first matmul)
                        # and closed once (stop on its last)
                        ps = dwps.tile([P, 4, 256], F32, tag="dw")
                        for j in range(nb):
                            gi = boff + j
                            out = ps[:, j, 0:NB * W].rearrange(
                                "p (b w) -> p b w", b=NB)
                            for dx in range(KK):
                                nc.tensor.matmul(
                                    out, ts[:, gi, dx, :],
                                    xs[:, gi, :, dx:dx + W],
                                    start=(j % 2 == 0 and dx == 0),
                                    stop=(j % 2 == 1 and dx == KK - 1),
                                    skip_group_check=True)
                        a4 = pa.tile([P, 4, NB, W], BF16, tag="a_g")
                        nc.scalar.activation(
                            a4[:, 0:nb].rearrange("p j b w -> p j (b w)"),
                            ps[:, 0:nb, 0:NB * W], ACT.Relu)
                        pend.append((g0 + boff, nb, a4))
                        if len(pend) > 1:
                            emit_t1()
                        # spread ch-0 T2 + reduces over the ch-1 slabs;
                        # the remainder drains at the tail
                        if s >= NSLAB // 2 and cfg["stop_after"] != "a":
                            for _ in range(cfg["bq_pops"]):
                                pop_bq()
                        boff += nb
                while pend:
                    emit_t1()

            # ============ Phase B tail (ch=1) / C (SE) / D (pointwise)
            if cfg["stop_after"] != "a":
                with tc.tile_pool(name="pdps", bufs=cfg["pdbufs"],
                                  space="PSUM") as pdps, \
                     tc.tile_pool(name="t2x", bufs=1, space="PSUM") as t2x, \
                     tc.tile_pool(name="seps", bufs=1, space="PSUM") as seps, \
                     tc.tile_pool(name="sep", bufs=2) as sep, \
                     tc.tile_pool(name="pd", bufs=3) as pd:
                    # drain any work items not absorbed into phase A
                    while bq:
                        pop_bq()
                    # 3-deep t2 rotation in the tail (spare PSUM bank)
                    t2state["pools"] = [t2ps, t2x, t2ps]
                    # tail schedule. D(q) reads only its own q's pixels, so
                    # D(0) does not wait on B(1,q1). q0 copies on Act; q0
                    # reduces on DVE right behind them; early PW tiles and
                    # the B(1,q1) transposes keep PE fed while the reduces
                    # run; B(1,q1) copies + q1 reduces land on DVE/Pool
                    # during D(0).
                    flush_trees()
                    phase_b(1, 0, copy_eng="act")
                    reduce_b(1, 0)
                    reduce_b(1, 1)
                    head = {}
                    if cfg["stop_after"] not in ("ab", "abc") \
                            and cfg["pw_head"]:
                        for nt in range(2):
                            head[(0, 0, nt)] = pw_tile(0, 0, nt)
                            head[(1, 0, nt)] = pw_tile(1, 0, nt)
                    se_q(0, half=0)
                    reduce_b(1, 2)
                    reduce_b(1, 3)
                    phase_b(1, 1, copy_eng="alt")
                    se_q(0, half=1)
                    if cfg["stop_after"] in ("ab", "abc"):
                        reduce_q(1, 1)
                        se_q(1)
                    else:
                        phase_d_bp(0, pw_head=head)
                        phase_d_bp(1)
                        reduce_q(1, 1)
                        se_q(1)
                        phase_d_bp(2)
                        phase_d_bp(3)

        pers.release()
        cst.release()

    nc.compile()
    _BUILD_CACHE[key] = nc
    return nc


# ---------------------------------------------------------------- host prep
def prep_inputs(inputs, cfg_key=None):
    f32 = np.float32
    bf16 = ml_dtypes.bfloat16

    x = np.asarray(inputs["x"], f32)
    dw_w = np.asarray(inputs["dw_w"], f32)      # [C,1,5,5]
    dw_b = np.asarray(inputs["dw_b"], f32)
    bn1_g = np.asarray(inputs["bn1_g"], f32)
    bn1_b = np.asarray(inputs["bn1_b"], f32)
    bn1_m = np.asarray(inputs["bn1_m"], f32)
    bn1_v = np.asarray(inputs["bn1_v"], f32)
    pw_w = np.asarray(inputs["pw_w"], f32)      # [Cout, C]
    pw_b = np.asarray(inputs["pw_b"], f32)
    se_w1 = np.asarray(inputs["se_w1"], f32)    # [R, C]
    se_b1 = np.asarray(inputs["se_b1"], f32)
    se_w2 = np.asarray(inputs["se_w2"], f32)    # [Cout, R]
    se_b2 = np.asarray(inputs["se_b2"], f32)
    bn2_g = np.asarray(inputs["bn2_g"], f32)
    bn2_b = np.asarray(inputs["bn2_b"], f32)
    bn2_m = np.asarray(inputs["bn2_m"], f32)
    bn2_v = np.asarray(inputs["bn2_v"], f32)

    # ---- Toeplitz, expanded block-diagonal: [hin(112), g, dx, (cj,hout)],
    # scaled by the BN1 s1 factor; two extra rows carry the (t1+3) bias
    # (split into bf16 hi + lo parts) in the dx=2 column, matching the
    # all-ones rows 112/113 of the packed x.
    s1 = bn1_g / np.sqrt(bn1_v + EPS)
    t1 = s1 * (dw_b - bn1_m) + bn1_b
    hin = np.arange(H)[:, None]
    hout = np.arange(H)[None, :]
    D = hin - hout
    mask = np.abs(D) <= 2
    dyi = np.clip(D + 2, 0, 4)
    k = dw_w[:, 0] * s1[:, None, None]                            # [C, 5, 5]
    band = np.where(mask[None, :, :, None], k[:, dyi, :], 0.0)    # [C,28,28,5]
    band_r = band.reshape(G, 4, H, H, KK)           # [g, ci, hin, hout, dx]
    toep4 = np.zeros((4, H, G, KK, 4, H), f32)
    for ci in range(4):
        # [g, hin, hout, dx] -> [hin, g, dx, hout]
        toep4[ci, :, :, :, ci, :] = band_r[:, ci].transpose(1, 0, 3, 2)
    toep = np.zeros((KP, G, KK, P), f32)
    toep[:P] = toep4.reshape(P, G, KK, P)
    bias = np.repeat((t1 + 3.0).reshape(G, 4), H, axis=1)     # [G, 112]
    bias_hi = bias.astype(bf16).astype(f32)
    toep[P, :, 2, :] = bias_hi
    toep[P + 1, :, 2, :] = bias - bias_hi
    toep = toep.astype(bf16)

    # ---- packed f32 constants
    cpk = np.zeros((128, NCOLS), f32)
    se1lT = (se_w1.T / HW)                                    # [C, R]
    for ch in range(2):
        cpk[:120, NC_SE1L + R * ch:NC_SE1L + R * (ch + 1)] = \
            se1lT[ch * 120:(ch + 1) * 120]
    cpk[:R, NC_SE1B] = se_b1
    se2lT = se_w2.T                                           # [R, Cout]
    for mo in range(2):
        cpk[:R, NC_SE2L + 120 * mo:NC_SE2L + 120 * (mo + 1)] = \
            se2lT[:, mo * 120:(mo + 1) * 120]
    s2 = bn2_g / np.sqrt(bn2_v + EPS)
    cpk[:120, NC_SE2B:NC_SE2B + 2] = (se_b2 + 3.0).reshape(2, 120).T
    cpk[:120, NC_BN2S:NC_BN2S + 2] = s2.reshape(2, 120).T
    cpk[:120, NC_BN2SB:NC_BN2SB + 2] = (s2 * pw_b).reshape(2, 120).T
    cpk[:120, NC_BN2T:NC_BN2T + 2] = (bn2_b - bn2_m * s2).reshape(2, 120).T

    # ---- pointwise weights [120, kc, mo, 120] bf16
    pwT = pw_w.T                                              # [C, Cout]
    pwl = np.zeros((120, 2, 2, 120), f32)
    for kc in range(2):
        for mo in range(2):
            pwl[:, kc, mo, :] = pwT[kc * 120:(kc + 1) * 120,
                                    mo * 120:(mo + 1) * 120]
    pwl = pwl.astype(bf16)

    shared = {"toep": toep, "cpk": cpk, "pwl": pwl}
    in_maps = []
    for i in range(N_CORES):
        m = dict(shared)
        # x -> [ci, h, g, b, w] -> [114, G, NB, 32] padded bf16,
        # rows 112/113 all-ones (bias rows)
        xs = x[i * NB:(i + 1) * NB].reshape(NB, G, 4, H, W)
        xr = np.zeros((KP, G, NB, WP), f32)
        xr[:P].reshape(4, H, G, NB, WP)[:, :, :, :, 2:2 + W] = \
            xs.transpose(2, 3, 1, 0, 4)
        xr[P:] = 1.0
        m["xr"] = np.ascontiguousarray(xr).astype(bf16)
        in_maps.append(m)
    return in_maps


def kernel(**inputs):
    nc = build_nc()
    in_maps = prep_inputs(inputs)
    res = run_bass_kernel_spmd(nc, in_maps, list(range(N_CORES)))
    out = np.concatenate([res.results[i]["y"] for i in range(N_CORES)], axis=0)
    return out.astype(np.float32)


# revision 89
# speedup vs baseline: 1.0174x; 1.0174x over previous
"""Trainium2 Bass kernel for nn_DepthWiseSepConv (depthwise 5x5 + BN+hardswish
+ pointwise 1x1 + squeeze-excite gating + BN), data-parallel over batch on
8 NeuronCores.

Self-contained: hardcodes all shapes from the problem spec.

Per-core layout strategy (B_loc = 8 images per core), all-bf16 matmul path:
  - Host pre-packs x into [112=(4c,28h), 60g, 8b, 32w] bf16 with zero pad
    baked in, and the depthwise weights into expanded block-diagonal
    Toeplitz matrices [112=(4c,28hin), 60g, 5dx, 112=(4c,28hout)] bf16.
    Slab DMAs (6 groups each) keep transfers large and DMA count low
    (each DMA costs ~630ns on the shared HWDGE unit).
  - Depthwise conv per group: 5 PSUM-accumulated matmuls (one per kernel
    column dx) contracting (4c,28h_in).
  - BN1+relu(x+3) fused into the PSUM-draining activation (bf16 out);
    hardswish completes AFTER the T1 transpose so the PSUM->SBUF copy is
    absorbed into the min/mult ops (Pool min, DVE mult writing ActT).
  - T2 transposes to channel-major PWrhs[c, (b,h,w)] bf16 + DVE copies.
  - SE: DVE free-dim reduces (per ch, per 4-batch), two small matmuls,
    hardswish; gate FOLDED into the pointwise epilogue's per-partition
    scale/bias (sg = s2*g, sbg = s2*pw_b*g + t2), so phase D is just
    matmul + one Activation + batched stores (b-pairs).
"""

import sys

sys.path.insert(0, "/opt/trn_rl_repo")

import numpy as np
import ml_dtypes

import concourse.bass as bass
import concourse.mybir as mybir
import concourse.tile as tile
from concourse import bacc
from concourse.bass_utils import run_bass_kernel_spmd
from concourse.masks import make_identity

# ---------------------------------------------------------------- constants
N_CORES = 8
B, C, H, W = 64, 240, 28, 28
NB = B // N_CORES          # images per core
KK = 5                      # depthwise kernel size
G = C // 4                  # channel groups of 4 -> 60
R = 60                      # SE reduction dim
Cout = 240
HW = H * W                  # 784
WP = 32                     # padded W in SBUF x tiles (w in [-2, 30))
P = 112                     # output partitions for DW: (4c, 28h)
KP = 114                    # contraction partitions: (4c,28h) + 2 bias rows
                            # (bias split hi+lo across two bf16 rows)
EPS = 1e-5

CFG = {
    "slab": 10,         # groups per x/toep DMA (aligned to ch halves)
    "nrot": 4,          # rotation depth of slab buffers
    "dwbufs": 2,        # DW psum pool depth (quad tiles, 2 banks each)
    "t1bufs": 2,
    "t2bufs": 2,
    "pdbufs": 4,
    "store_split": False,   # y stores per (b, mo) instead of b-pairs
    # engine for the hardswish min op; must read PSUM, so pool is NOT
    # HW-legal (GPSIMD cannot access PSUM — walrus birverifier rejects it)
    "min_engine": "dve",
    "red_split": True,      # split SE reduce per (ch, q)
    "epi_dve": 4,           # every k-th D epilogue goes to DVE (1000=never)
    "bq_pops": 2,           # ch-0 tail work items emitted per ch-1 block
    "warmup": 10,           # dummy PE transposes during cold-start DMA wait
    "pw_head": False,        # emit 4 early PW tiles before SE(0)
    "stop_after": "",       # debug: ""|"a"|"ab"|"abc"
}

F32 = mybir.dt.float32
BF16 = mybir.dt.bfloat16
AL = mybir.AluOpType
ACT = mybir.ActivationFunctionType

# packed f32 const columns
NC_BN1S = 0          # [112, 60]
NC_BN1B = 60         # [112, 60]
NC_SE1L = 120        # [120, 2*60]
NC_SE1B = 240        # [60, 1]
NC_SE2L = 241        # [60, 2*120]
NC_SE2B = 481        # [120, 2]
NC_BN2S = 483        # [120, 2]
NC_BN2SB = 485       # [120, 2]
NC_BN2T = 487        # [120, 2]
NCOLS = 489


# ---------------------------------------------------------------- builder
_BUILD_CACHE = {}


def build_nc(cfg_key=None):
    cfg = dict(CFG)
    if cfg_key is not None:
        cfg.update(cfg_key)
    key = tuple(sorted(cfg.items()))
    if key in _BUILD_CACHE:
        return _BUILD_CACHE[key]

    SLAB = cfg["slab"]
    NSLAB = G // SLAB
    NROT = cfg["nrot"]

    nc = bacc.Bacc("TRN2", target_bir_lowering=False, debug=False,
                   num_devices=N_CORES)

    x_p = nc.declare_dram_parameter("xr", [KP, G, NB, WP], BF16,
                                    isOutput=False)
    toep_p = nc.declare_dram_parameter("toep", [KP, G, KK, P], BF16,
                                       isOutput=False)
    # block-0 x + toep packed together: one cold-start DMA instead of two
    NBOOT = 2 * NB * WP + 2 * KK * P
    boot_p = nc.declare_dram_parameter("boot", [KP, NBOOT], BF16,
                                       isOutput=False)
    cpk_p = nc.declare_dram_parameter("cpk", [128, NCOLS], F32, isOutput=False)
    pwl_p = nc.declare_dram_parameter("pwl", [120, 2, 2, 120], BF16,
                                      isOutput=False)
    y_p = nc.declare_dram_parameter("y", [NB, Cout, H, W], F32, isOutput=True)

    with tile.TileContext(nc) as tc:
        cst = tc.alloc_tile_pool(name="cst", bufs=1)
        pers = tc.alloc_tile_pool(name="pers", bufs=1)

        # first compute slab's inputs go first (PE cold start gates on them);
        # pwl is not needed until phase D
        xs0 = None
        cpk = cst.tile([128, NCOLS], F32)
        pwl_sb = cst.tile([120, 2, 2, 120], BF16)
        ident = cst.tile([128, 128], BF16)

        def cc(col, ncol=1, rows=128):
            return cpk[0:rows, col:col + ncol]

        # persistent activation buffers
        # ActT[ch]: [(4b,28w)=112, 2q, (30gl, 4c, 28h) = 3360]
        ActT = [pers.tile([P, 2, 30 * P], BF16, name=f"actt_{ch}")
                for ch in range(2)]
        # PWrhs[ch]: [c=120, 8b, 28h, 28w]
        PWrhs = [pers.tile([120, NB, H, W], BF16, name=f"pwrhs_{ch}")
                 for ch in range(2)]
        PWflat = [t[:].rearrange("p b h w -> p (b h w)") for t in PWrhs]
        s_sb = [pers.tile([120, NB], F32, name=f"s_{ch}") for ch in range(2)]
        g_t = pers.tile([120, 2, NB], F32, name="gate")
        sg = pers.tile([120, 2, NB], F32, name="sg")
        sbg = pers.tile([120, 2, NB], F32, name="sbg")

        # slab rotation buffers
        xb = [pers.tile([KP, SLAB, NB, WP], BF16, name=f"x_rot{i}")
              for i in range(NROT)]
        tb = [pers.tile([KP, SLAB, KK, P], BF16, name=f"t_rot{i}")
              for i in range(NROT)]
        boot_sb = cst.tile([KP, NBOOT], BF16)
        bxv = boot_sb[:, 0:2 * NB * WP].rearrange(
            "p (g b w) -> p g b w", g=2, b=NB)
        btv = boot_sb[:, 2 * NB * WP:].rearrange(
            "p (g k m) -> p g k m", g=2, k=KK)

        min_eng = nc.gpsimd if cfg["min_engine"] == "pool" else nc.vector

        t2state = {"pools": None, "i": 0}

        def phase_b_chunk(ch, q, h0, copy_eng=None):
            """T2: transpose ActT (ch, q, 4 h-rows) into PWrhs[ch]."""
            v = ActT[ch][:].rearrange("p q (gl c e) -> p q gl c e",
                                      gl=30, c=4)
            pools = t2state["pools"] or [t2ps]
            pool = pools[t2state["i"] % len(pools)]
            t2state["i"] += 1
            tp2 = pool.tile([120, 4, P], BF16, tag="t2")
            for hi in range(4):
                nc.tensor.transpose(tp2[:, hi, :],
                                    v[:, q, :, :, h0 + hi],
                                    ident[:P, :P])
            dst = PWrhs[ch][:, 4 * q:4 * q + 4, h0:h0 + 4, :]
            src = tp2[:].rearrange("p hh (b w) -> p b hh w", b=4)
            if copy_eng == "alt":
                copy_eng = "act" if (h0 // 4) % 2 == 0 else None
            if copy_eng == "act":
                nc.scalar.copy(dst, src)
            else:
                nc.vector.tensor_copy(dst, src)

        def phase_b(ch, q, copy_eng=None):
            for h0 in range(0, H, 4):
                phase_b_chunk(ch, q, h0, copy_eng)

        rtree = pers.tile([120, NB, 98], F32, name="rtree")
        tree_pend = []

        def reduce_b(ch, b, pool_tree=False):
            row = PWflat[ch].rearrange("p (b f) -> p b f", b=NB)[:, b, :]
            if pool_tree:
                # log-fold on the (otherwise idle) Pool engine — it cannot
                # touch PSUM, but these are all-SBUF. The small DVE finishing
                # reduce is DEFERRED (flush_trees) so a slow tree never
                # head-blocks DVE's in-order queue mid-phase-A.
                f1 = pa.tile([120, 392], F32, tag="rf1")
                nc.gpsimd.tensor_tensor(f1[:], row[:, 0:392], row[:, 392:784],
                                        AL.add)
                f2 = pa.tile([120, 196], F32, tag="rf2")
                nc.gpsimd.tensor_tensor(f2[:], f1[:, 0:196], f1[:, 196:392],
                                        AL.add)
                nc.gpsimd.tensor_tensor(rtree[:, b, :], f2[:, 0:98],
                                        f2[:, 98:196], AL.add)
                tree_pend.append((ch, b))
            else:
                nc.vector.tensor_reduce(s_sb[ch][:, b:b + 1], row,
                                        mybir.AxisListType.X, AL.add)

        def flush_trees():
            while tree_pend:
                ch, b = tree_pend.pop(0)
                nc.vector.tensor_reduce(s_sb[ch][:, b:b + 1], rtree[:, b, :],
                                        mybir.AxisListType.X, AL.add)

        def reduce_q(ch, q, pool_last=False):
            if cfg["red_split"]:
                for i, b in enumerate(range(4 * q, 4 * q + 4)):
                    reduce_b(ch, b, pool_tree=(pool_last and i == 0))
            else:
                nc.vector.tensor_reduce(
                    s_sb[ch][:, 4 * q:4 * q + 4],
                    PWflat[ch].rearrange("p (b f) -> p b f", b=NB)[
                        :, 4 * q:4 * q + 4, :],
                    mybir.AxisListType.X, AL.add)

        def se_q(q, half=None):
            """SE gate + fold into per-(mo,b) scale/bias for phase D.

            half=0/1 computes just a b-pair (lets D start as soon as the
            first two reduces land)."""
            if half is None:
                b0, nb = 4 * q, 4
            else:
                b0, nb = 4 * q + 2 * half, 2
            bs = slice(b0, b0 + nb)
            ps1 = seps.tile([120, 4], F32, tag="se")
            for ch in range(2):
                nc.tensor.matmul(ps1[0:R, 0:nb], cc(NC_SE1L + R * ch, R, 120),
                                 s_sb[ch][:, bs],
                                 start=(ch == 0), stop=(ch == 1))
            h1 = sep.tile([R, 4], F32, tag="seh")
            nc.scalar.activation(h1[:, 0:nb], ps1[0:R, 0:nb], ACT.Relu,
                                 bias=cc(NC_SE1B, 1, R))
            for mo in range(2):
                ps2 = seps.tile([120, 4], F32, tag="se")
                nc.tensor.matmul(ps2[:, 0:nb], cc(NC_SE2L + 120 * mo, 120, R),
                                 h1[:, 0:nb], start=True, stop=True)
                a2 = sep.tile([120, 4], F32, tag="sea")
                nc.scalar.activation(a2[:, 0:nb], ps2[:, 0:nb], ACT.Relu,
                                     bias=cc(NC_SE2B + mo, 1, 120))
                m2 = sep.tile([120, 4], F32, tag="sem")
                nc.vector.tensor_scalar(m2[:, 0:nb], a2[:, 0:nb],
                                        1.0 / 6.0, 1.0, AL.mult, AL.min)
                nc.vector.scalar_tensor_tensor(g_t[:, mo, bs], a2[:, 0:nb],
                                               3.0, m2[:, 0:nb],
                                               AL.subtract, AL.mult)
                nc.vector.tensor_scalar(sg[:, mo, bs], g_t[:, mo, bs],
                                        cc(NC_BN2S + mo, 1, 120), None,
                                        AL.mult)
                nc.vector.scalar_tensor_tensor(
                    sbg[:, mo, bs], g_t[:, mo, bs],
                    cc(NC_BN2SB + mo, 1, 120),
                    cc(NC_BN2T + mo, 1, 120).to_broadcast((120, nb)),
                    AL.mult, AL.add)

        def pw_tile(mo, b, nt):
            ps = pdps.tile([120, 392], F32, tag="pw")
            off = b * HW + nt * 392
            for kc in range(2):
                nc.tensor.matmul(ps[:], pwl_sb[:, kc, mo, :],
                                 PWflat[kc][:, off:off + 392],
                                 start=(kc == 0), stop=(kc == 1))
            return ps

        def epilogue(o_ap, ps, mo, b, eng):
            """o = ps*sg + sbg; alternates Act/DVE to balance the D phase."""
            if eng == "dve":
                nc.vector.scalar_tensor_tensor(
                    o_ap, ps[:], sg[:, mo, b:b + 1],
                    sbg[:, mo, b:b + 1].to_broadcast((120, 392)),
                    AL.mult, AL.add)
            else:
                nc.scalar.activation(o_ap, ps[:], ACT.Identity,
                                     bias=sbg[:, mo, b:b + 1],
                                     scale=sg[:, mo, b:b + 1])

        def phase_d(q, finer_stores=False, pw_head=None):
            for bp in (2 * q, 2 * q + 1):
                phase_d_bp(bp, pw_head)

        def phase_d_bp(bp, pw_head=None):
            """Pointwise conv + fused gate/BN2 epilogue + stores, one
            b-pair."""
            ei = 0
            if True:
                last = (bp >= 2)
                for mo in range(2):
                    o2 = pd.tile([120, 2, HW], F32, tag="o2")
                    for bi in range(2):
                        b = 2 * bp + bi
                        for nt in range(2):
                            if pw_head and (mo, b, nt) in pw_head:
                                ps = pw_head.pop((mo, b, nt))
                            else:
                                ps = pw_tile(mo, b, nt)
                            # last pair: alternate engines so the final
                            # epilogues run in parallel
                            eng = ("dve" if ei % 2 else "act") if last else \
                                ("dve" if ei % cfg["epi_dve"] == 0
                                 else "act")
                            epilogue(o2[:, bi, nt * 392:(nt + 1) * 392], ps,
                                     mo, b, eng)
                            ei += 1
                        if last:
                            # final pair: store per image so the kernel-end
                            # drain isn't gated by one big DMA
                            y_ap = y_p[b, mo * 120:(mo + 1) * 120] \
                                .rearrange("c h w -> c (h w)")
                            nc.sync.dma_start(y_ap, o2[:, bi])
                    if not last:
                        y_ap = y_p[2 * bp:2 * bp + 2,
                                   mo * 120:(mo + 1) * 120].rearrange(
                            "b c h w -> c b (h w)")
                        nc.sync.dma_start(y_ap, o2[:])

        with tc.tile_pool(name="t2ps", bufs=cfg["t2bufs"], space="PSUM") \
                as t2ps, \
             tc.tile_pool(name="pa", bufs=3) as pa:

            # ============ Phase A: depthwise + relu + T1 + hardswish
            # Groups are processed in BLOCKS of 4 (pairs at the channel-half
            # edges) sharing one psum tile / one Act / one min / one
            # hardswish-mult (amortizes per-op access tax). BN1 is folded
            # into the toep weights + two all-ones x rows whose dx=2 weight
            # column carries the bias hi/lo, so the Act is a pure Relu with
            # no per-group scale/bias. T1 for block k-1 is emitted after
            # block k's matmuls (software pipelining). T2 chunks + reduces
            # for channel half 0 are spread across the second half's slabs.
            pend = []   # (g_start, nb, a_tile) awaiting T1 emission

            def emit_t1():
                g0, nb, a4 = pend.pop(0)
                ch, gl = (0, g0) if g0 < 30 else (1, g0 - 30)
                tp = t1ps.tile([P, 2, 4, P], BF16, tag="t1")
                for j in range(nb):
                    for q in range(2):
                        nc.tensor.transpose(tp[:, q, j, :],
                                            a4[:, j, 4 * q:4 * q + 4, :],
                                            ident[:P, :P])
                m = pa.tile([P, 2, 4, P], BF16, tag="m_g")
                min_eng.tensor_scalar(m[:, :, 0:nb, :], tp[:, :, 0:nb, :],
                                      1.0 / 6.0, 1.0, AL.mult, AL.min)
                nc.vector.scalar_tensor_tensor(
                    ActT[ch][:, :, gl * P:(gl + nb) * P].rearrange(
                        "p q (j e) -> p q j e", j=nb),
                    tp[:, :, 0:nb, :], 3.0, m[:, :, 0:nb, :],
                    AL.subtract, AL.mult)

            with tc.tile_pool(name="dwps", bufs=cfg["dwbufs"],
                              space="PSUM") as dwps, \
                 tc.tile_pool(name="t1ps", bufs=cfg["t1bufs"],
                              space="PSUM") as t1ps:
                # work items (T2 chunks + SE reduces for channel half 0)
                # interleaved into the second half's slabs
                bq = []
                for q in range(2):
                    bq += [("b", 0, q, h0) for h0 in range(0, H, 4)]
                    bq += [("r", 0, b, None) for b in range(4 * q, 4 * q + 4)]

                def pop_bq():
                    if bq:
                        kind, ch_, i1, i2 = bq.pop(0)
                        if kind == "b":
                            # copies on Act: a DVE copy here would head-block
                            # the min/stt chain that recycles t1ps for PE
                            phase_b_chunk(ch_, i1, i2, copy_eng="act")
                        else:
                            reduce_b(ch_, i1, pool_tree=True)

                for s in range(NSLAB):
                    xs = xb[s % NROT]
                    ts = tb[s % NROT]
                    g0 = SLAB * s
                    # pair-first on the slabs that start a channel half
                    # (cheap cold-start block), pair-last otherwise
                    blocks = [2, 4, 4] if s % (NSLAB // 2) == 0 else [4, 4, 2]
                    if s == 0:
                        # cold start: block 0's x+toep arrive in ONE packed
                        # DMA; consts wait until slab 2 (nothing in phase A
                        # reads them)
                        nc.sync.dma_start(boot_sb[:], boot_p[:])
                        nc.sync.dma_start(xs[:, 2:SLAB], x_p[:, 2:SLAB])
                        nc.sync.dma_start(ts[:, 2:SLAB], toep_p[:, 2:SLAB])
                        make_identity(nc, ident[:])
                        if cfg["warmup"]:
                            # ramp the PE p-state while the first slab's
                            # DMAs are in flight (results never read)
                            wt = t1ps.tile([P, 2, 4, P], BF16, tag="t1")
                            for _ in range(cfg["warmup"]):
                                nc.tensor.transpose(wt[:, 0, 0, :],
                                                    ident[:P, :P],
                                                    ident[:P, :P])
                    else:
                        # half-slab granularity: blocks only wait on the
                        # half they read, smoothing DMA arrival
                        h = SLAB // 2
                        for hs in range(2):
                            bs = slice(hs * h, (hs + 1) * h)
                            gs = slice(g0 + hs * h, g0 + (hs + 1) * h)
                            nc.sync.dma_start(xs[:, bs], x_p[:, gs])
                            nc.sync.dma_start(ts[:, bs], toep_p[:, gs])
                        if s == 2:
                            nc.sync.dma_start(cpk[:], cpk_p[:])
                            nc.sync.dma_start(pwl_sb[:], pwl_p[:])
                    boff = 0
                    for nb in blocks:
                        # regions padded to 256 f32 (half a PSUM bank) so no
                        # region crosses a bank; each bank's accumulation
                        # group is opened once (start on its first matmul)
                        # and closed once (stop on its last)
                        ps = dwps.tile([P, 4, 256], F32, tag="dw")
                        booted = (s == 0 and boff == 0)
                        for j in range(nb):
                            gi = boff + j
                            tsv = btv[:, j] if booted else ts[:, gi]
                            xsv = bxv[:, j] if booted else xs[:, gi]
                            out = ps[:, j, 0:NB * W].rearrange(
                                "p (b w) -> p b w", b=NB)
                            for dx in range(KK):
                                nc.tensor.matmul(
                                    out, tsv[:, dx, :],
                                    xsv[:, :, dx:dx + W],
                                    start=(j % 2 == 0 and dx == 0),
                                    stop=(j % 2 == 1 and dx == KK - 1),
                                    skip_group_check=True)
                        a4 = pa.tile([P, 4, NB, W], BF16, tag="a_g")
                        nc.scalar.activation(
                            a4[:, 0:nb].rearrange("p j b w -> p j (b w)"),
                            ps[:, 0:nb, 0:NB * W], ACT.Relu)
                        pend.append((g0 + boff, nb, a4))
                        if len(pend) > 1:
                            emit_t1()
                        # spread ch-0 T2 + reduces over the ch-1 slabs;
                        # the remainder drains at the tail
                        if s >= NSLAB // 2 and cfg["stop_after"] != "a":
                            for _ in range(cfg["bq_pops"]):
                                pop_bq()
                        boff += nb
                while pend:
                    emit_t1()

            # ============ Phase B tail (ch=1) / C (SE) / D (pointwise)
            if cfg["stop_after"] != "a":
                with tc.tile_pool(name="pdps", bufs=cfg["pdbufs"],
                                  space="PSUM") as pdps, \
                     tc.tile_pool(name="t2x", bufs=1, space="PSUM") as t2x, \
                     tc.tile_pool(name="seps", bufs=1, space="PSUM") as seps, \
                     tc.tile_pool(name="sep", bufs=2) as sep, \
                     tc.tile_pool(name="pd", bufs=3) as pd:
                    # drain any work items not absorbed into phase A
                    while bq:
                        pop_bq()
                    # 3-deep t2 rotation in the tail (spare PSUM bank)
                    t2state["pools"] = [t2ps, t2x, t2ps]
                    # tail schedule. D(q) reads only its own q's pixels, so
                    # D(0) does not wait on B(1,q1). q0 copies on Act; q0
                    # reduces on DVE right behind them; early PW tiles and
                    # the B(1,q1) transposes keep PE fed while the reduces
                    # run; B(1,q1) copies + q1 reduces land on DVE/Pool
                    # during D(0).
                    flush_trees()
                    phase_b(1, 0, copy_eng="act")
                    reduce_b(1, 0)
                    reduce_b(1, 1)
                    head = {}
                    if cfg["stop_after"] not in ("ab", "abc") \
                            and cfg["pw_head"]:
                        for nt in range(2):
                            head[(0, 0, nt)] = pw_tile(0, 0, nt)
                            head[(1, 0, nt)] = pw_tile(1, 0, nt)
                    se_q(0, half=0)
                    reduce_b(1, 2)
                    reduce_b(1, 3)
                    phase_b(1, 1, copy_eng="alt")
                    se_q(0, half=1)
                    if cfg["stop_after"] in ("ab", "abc"):
                        reduce_q(1, 1)
                        se_q(1)
                    else:
                        phase_d_bp(0, pw_head=head)
                        phase_d_bp(1)
                        reduce_q(1, 1)
                        se_q(1)
                        phase_d_bp(2)
                        phase_d_bp(3)

        pers.release()
        cst.release()

    nc.compile()
    _BUILD_CACHE[key] = nc
    return nc


# ---------------------------------------------------------------- host prep
def prep_inputs(inputs, cfg_key=None):
    f32 = np.float32
    bf16 = ml_dtypes.bfloat16

    x = np.asarray(inputs["x"], f32)
    dw_w = np.asarray(inputs["dw_w"], f32)      # [C,1,5,5]
    dw_b = np.asarray(inputs["dw_b"], f32)
    bn1_g = np.asarray(inputs["bn1_g"], f32)
    bn1_b = np.asarray(inputs["bn1_b"], f32)
    bn1_m = np.asarray(inputs["bn1_m"], f32)
    bn1_v = np.asarray(inputs["bn1_v"], f32)
    pw_w = np.asarray(inputs["pw_w"], f32)      # [Cout, C]
    pw_b = np.asarray(inputs["pw_b"], f32)
    se_w1 = np.asarray(inputs["se_w1"], f32)    # [R, C]
    se_b1 = np.asarray(inputs["se_b1"], f32)
    se_w2 = np.asarray(inputs["se_w2"], f32)    # [Cout, R]
    se_b2 = np.asarray(inputs["se_b2"], f32)
    bn2_g = np.asarray(inputs["bn2_g"], f32)
    bn2_b = np.asarray(inputs["bn2_b"], f32)
    bn2_m = np.asarray(inputs["bn2_m"], f32)
    bn2_v = np.asarray(inputs["bn2_v"], f32)

    # ---- Toeplitz, expanded block-diagonal: [hin(112), g, dx, (cj,hout)],
    # scaled by the BN1 s1 factor; two extra rows carry the (t1+3) bias
    # (split into bf16 hi + lo parts) in the dx=2 column, matching the
    # all-ones rows 112/113 of the packed x.
    s1 = bn1_g / np.sqrt(bn1_v + EPS)
    t1 = s1 * (dw_b - bn1_m) + bn1_b
    hin = np.arange(H)[:, None]
    hout = np.arange(H)[None, :]
    D = hin - hout
    mask = np.abs(D) <= 2
    dyi = np.clip(D + 2, 0, 4)
    k = dw_w[:, 0] * s1[:, None, None]                            # [C, 5, 5]
    band = np.where(mask[None, :, :, None], k[:, dyi, :], 0.0)    # [C,28,28,5]
    band_r = band.reshape(G, 4, H, H, KK)           # [g, ci, hin, hout, dx]
    toep4 = np.zeros((4, H, G, KK, 4, H), f32)
    for ci in range(4):
        # [g, hin, hout, dx] -> [hin, g, dx, hout]
        toep4[ci, :, :, :, ci, :] = band_r[:, ci].transpose(1, 0, 3, 2)
    toep = np.zeros((KP, G, KK, P), f32)
    toep[:P] = toep4.reshape(P, G, KK, P)
    bias = np.repeat((t1 + 3.0).reshape(G, 4), H, axis=1)     # [G, 112]
    bias_hi = bias.astype(bf16).astype(f32)
    toep[P, :, 2, :] = bias_hi
    toep[P + 1, :, 2, :] = bias - bias_hi
    toep = toep.astype(bf16)

    # ---- packed f32 constants
    cpk = np.zeros((128, NCOLS), f32)
    se1lT = (se_w1.T / HW)                                    # [C, R]
    for ch in range(2):
        cpk[:120, NC_SE1L + R * ch:NC_SE1L + R * (ch + 1)] = \
            se1lT[ch * 120:(ch + 1) * 120]
    cpk[:R, NC_SE1B] = se_b1
    se2lT = se_w2.T                                           # [R, Cout]
    for mo in range(2):
        cpk[:R, NC_SE2L + 120 * mo:NC_SE2L + 120 * (mo + 1)] = \
            se2lT[:, mo * 120:(mo + 1) * 120]
    s2 = bn2_g / np.sqrt(bn2_v + EPS)
    cpk[:120, NC_SE2B:NC_SE2B + 2] = (se_b2 + 3.0).reshape(2, 120).T
    cpk[:120, NC_BN2S:NC_BN2S + 2] = s2.reshape(2, 120).T
    cpk[:120, NC_BN2SB:NC_BN2SB + 2] = (s2 * pw_b).reshape(2, 120).T
    cpk[:120, NC_BN2T:NC_BN2T + 2] = (bn2_b - bn2_m * s2).reshape(2, 120).T

    # ---- pointwise weights [120, kc, mo, 120] bf16
    pwT = pw_w.T                                              # [C, Cout]
    pwl = np.zeros((120, 2, 2, 120), f32)
    for kc in range(2):
        for mo in range(2):
            pwl[:, kc, mo, :] = pwT[kc * 120:(kc + 1) * 120,
                                    mo * 120:(mo + 1) * 120]
    pwl = pwl.astype(bf16)

    shared = {"toep": toep, "cpk": cpk, "pwl": pwl}
    in_maps = []
    for i in range(N_CORES):
        m = dict(shared)
        # x -> [ci, h, g, b, w] -> [114, G, NB, 32] padded bf16,
        # rows 112/113 all-ones (bias rows)
        xs = x[i * NB:(i + 1) * NB].reshape(NB, G, 4, H, W)
        xr = np.zeros((KP, G, NB, WP), f32)
        xr[:P].reshape(4, H, G, NB, WP)[:, :, :, :, 2:2 + W] = \
            xs.transpose(2, 3, 1, 0, 4)
        xr[P:] = 1.0
        xr = np.ascontiguousarray(xr).astype(bf16)
        m["xr"] = xr
        # block 0's x + toep packed into one cold-start tensor
        m["boot"] = np.ascontiguousarray(np.concatenate(
            [xr[:, 0:2].reshape(KP, -1), toep[:, 0:2].reshape(KP, -1)],
            axis=1))
        in_maps.append(m)
    return in_maps


def kernel(**inputs):
    nc = build_nc()
    in_maps = prep_inputs(inputs)
    res = run_bass_kernel_spmd(nc, in_maps, list(range(N_CORES)))
    out = np.concatenate([res.results[i]["y"] for i in range(N_CORES)], axis=0)
    return out.astype(np.float32)
